# revision 4
# baseline (speedup 1.0000x reference)
"""BitLinear (ternary-quantized linear) kernel for 8 Trainium2 NeuronCores.

Reference computation:
    w_mean = mean(|W|)                       (global scalar over the full W)
    W_q    = clip(round(W / (w_mean+1e-5)), -1, 1)   in {-1, 0, 1}
    out    = x @ (W_q * w_mean * scale[0]).T

Sharding (column parallel): W is split along out_features across the 8 cores
(2048 rows each); x is replicated; each core produces out[:, shard] and the
host concatenates shards.

Device-side work per core:
  Phase A: partial sum of |W_shard| (DVE reduce) -> cross-partition reduce on
           GpSimd -> AllReduce(add) over the 8 cores -> w_mean, threshold
           h = 0.5*(w_mean+1e-5), out-scale so = 0.5*w_mean*scale[0]
           (the extra 0.5 dequantizes the {-2,0,2} ternary encoding below).
  Phase B: quantize W -> ternary in {-2, 0, +2} bf16, resident in SBUF
           K-major [128k, KT, N].  The work is spread over three engines so
           it keeps pace with the second W DMA pass:
             even chunks: ACT sign(w-h), ACT sign(w+h), DVE add  -> {-2,0,2}
             odd  chunks: GpSimd (w>=h)*2, GpSimd (w<=-h)*2, DVE sub
  Phase C: tiled matmul: stationary = x^T tiles (bf16), moving = W_q strips,
           fp32 PSUM accumulation over K, eviction scales by so on DVE.
           Matmuls of the first two PSUM groups overlap phase B.

Host marshaling: x is cast to bf16 and laid out K-major ([mt, kt, k, m]);
W is laid out K-major f32 per shard ([kt, k, n]). All FLOPs of the reference
computation (mean, quantize, matmul, scaling) happen on device.
"""

import numpy as np
import ml_dtypes

CORES = 8
B, S, DIN, DOUT = 4, 2048, 4096, 16384

_LDW_PATCHED = False


def _enable_ldw_opt():
    """Flip walrus --enable-ldw-opt to true: dedupes back-to-back LDWEIGHTS
    for matmuls that reuse the same stationary operand."""
    global _LDW_PATCHED
    if _LDW_PATCHED:
        return
    import concourse.bass_utils as bu
    orig = bu.run_command

    def patched(argv, **kw):
        argv = ["--enable-ldw-opt=true" if a == "--enable-ldw-opt=false" else a
                for a in argv]
        return orig(argv, **kw)

    bu.run_command = patched
    _LDW_PATCHED = True


M_TOK = B * S              # 8192 tokens
N_SHARD = DOUT // CORES    # 2048 out-features per core
MSLAB = 256                # tokens per x slab (2 stationary subtiles of 128)


def build_nc(cores=CORES, M=M_TOK, K=DIN, N=N_SHARD, n_weight_total=DOUT * DIN):
    """Build the (uncompiled-IR) Bass module for one SPMD core program."""
    import concourse.bacc as bacc
    import concourse.tile as tile
    import concourse.mybir as mybir
    import concourse.bass_isa as bass_isa

    f32 = mybir.dt.float32
    bf16 = mybir.dt.bfloat16
    X = mybir.AxisListType.X
    ADD = mybir.AluOpType.add
    MULT = mybir.AluOpType.mult
    ISGE = mybir.AluOpType.is_ge
    ISLE = mybir.AluOpType.is_le
    SIGN = mybir.ActivationFunctionType.Sign

    KT = K // 128
    NSW = 512               # matmul moving-strip width (PSUM bank limit)
    NS = N // NSW
    MT = M // MSLAB
    MS = MSLAB // 128
    QCH = 512               # quantize column chunk
    NQ = N // QCH
    inv_nw = 1.0 / float(n_weight_total)

    nc = bacc.Bacc("TRN2", target_bir_lowering=False, debug=False,
                   num_devices=cores)
    xt = nc.dram_tensor("xt", [MT, KT, 128, MSLAB], bf16, kind="ExternalInput")
    wt = nc.dram_tensor("wt", [KT, 128, N], f32, kind="ExternalInput")
    sc = nc.dram_tensor("scale", [1], f32, kind="ExternalInput")
    out = nc.dram_tensor("out", [M, N], f32, kind="ExternalOutput")

    with tile.TileContext(nc) as tc:
        with tc.tile_pool(name="const", bufs=1) as const, \
             tc.tile_pool(name="wqp", bufs=1) as wqp, \
             tc.tile_pool(name="wload", bufs=3) as wload, \
             tc.tile_pool(name="qtmp", bufs=2) as qtmp, \
             tc.tile_pool(name="xload", bufs=2) as xload, \
             tc.tile_pool(name="otp", bufs=2) as otp, \
             tc.tile_pool(name="pp", bufs=8 * 512 // NSW, space="PSUM") as pp, \
             tc.tile_pool(name="dram", bufs=1, space="DRAM") as dram:

            # ---------- Phase A: global mean(|W|) ----------
            partials = const.tile([128, KT], f32)
            for kt in range(KT):
                wa = wload.tile([128, N], f32, tag="w")
                eng = nc.sync if kt % 2 == 0 else nc.scalar
                eng.dma_start(out=wa[:], in_=wt[kt])
                nc.vector.tensor_reduce(partials[:, kt:kt + 1], wa[:], axis=X,
                                        op=ADD, apply_absolute_value=True)
            p1 = const.tile([128, 1], f32)
            nc.vector.tensor_reduce(p1[:], partials[:], axis=X, op=ADD)
            # cross-partition reduce on GpSimd: avoids an fp32 PE matmul,
            # which would disable fast-weight-load for the bf16 matmuls.
            tot_all = const.tile([128, 1], f32)
            nc.gpsimd.partition_all_reduce(tot_all[:], p1[:], 128,
                                           bass_isa.ReduceOp.add)
            tot_sb = tot_all[0:1, 0:1]

            cc_in = dram.tile([1, 1], f32)
            cc_space = "Shared" if cores > 4 else "Local"
            cc_out = dram.tile([1, 1], f32, addr_space=cc_space)
            nc.sync.dma_start(out=cc_in[:], in_=tot_sb[:])
            nc.gpsimd.collective_compute(
                "AllReduce", ADD,
                replica_groups=[list(range(cores))],
                ins=[cc_in.opt()], outs=[cc_out.opt()],
            )
            gsum = const.tile([1, 1], f32)
            nc.sync.dma_start(out=gsum[:], in_=cc_out[:])

            mean1 = const.tile([1, 1], f32)
            nc.scalar.mul(mean1[:], gsum[:], inv_nw)
            # h = 0.5 * (mean + 1e-5)
            me1 = const.tile([1, 1], f32)
            nc.vector.tensor_scalar(me1[:], mean1[:], 1e-5, None, ADD)
            h1 = const.tile([1, 1], f32)
            nc.vector.tensor_scalar(h1[:], me1[:], 0.5, None, MULT)
            sc_sb = const.tile([1, 1], f32)
            nc.sync.dma_start(out=sc_sb[:], in_=sc[:].rearrange("(a b) -> a b", b=1))
            # so = 0.5 * mean * scale  (the 0.5 dequantizes the {-2,0,2} code)
            mh1 = const.tile([1, 1], f32)
            nc.vector.tensor_scalar(mh1[:], mean1[:], 0.5, None, MULT)
            so1 = const.tile([1, 1], f32)
            nc.vector.tensor_mul(so1[:], mh1[:], sc_sb[:])
            h128 = const.tile([128, 1], f32)
            nc.gpsimd.partition_broadcast(h128[:], h1[:])
            h128n = const.tile([128, 1], f32)
            nc.vector.tensor_scalar(h128n[:], h128[:], -1.0, None, MULT)
            so128 = const.tile([128, 1], f32)
            nc.gpsimd.partition_broadcast(so128[:], so1[:])

            # ---------- Phase B: quantize W -> ternary {-2,0,2} bf16 ----------
            # Spread across ACT + GpSimd (compares) and DVE (combine) so the
            # per-tile cost stays below the 2.9us DMA pace and matmuls of the
            # open PSUM groups can interleave.
            wq = wqp.tile([128, KT, N], bf16)
            for kt in range(KT):
                wb = wload.tile([128, N], f32, tag="w")
                eng = nc.sync if kt % 2 == 0 else nc.scalar
                eng.dma_start(out=wb[:], in_=wt[kt])
                for c in range(NQ):
                    sl = slice(c * QCH, (c + 1) * QCH)
                    if c % 2 == 0:
                        s1 = qtmp.tile([128, QCH], bf16, tag="qa")
                        nc.scalar.activation(s1[:], wb[:, sl], SIGN, bias=h128n[:])
                        s2 = qtmp.tile([128, QCH], bf16, tag="qb")
                        nc.scalar.activation(s2[:], wb[:, sl], SIGN, bias=h128[:])
                        nc.vector.tensor_add(wq[:, kt, sl], s1[:], s2[:])
                    else:
                        g1 = qtmp.tile([128, QCH], bf16, tag="qa")
                        nc.gpsimd.tensor_scalar(g1[:], wb[:, sl], h128[:], 2.0,
                                                ISGE, MULT)
                        g2 = qtmp.tile([128, QCH], bf16, tag="qb")
                        nc.gpsimd.tensor_scalar(g2[:], wb[:, sl], h128n[:], 2.0,
                                                ISLE, MULT)
                        nc.vector.tensor_sub(wq[:, kt, sl], g1[:], g2[:])

            # ---------- Phase C: out = x @ W_q^T * so ----------
            for mt in range(MT):
                xs = xload.tile([128, KT, MSLAB], bf16)
                nc.sync.dma_start(out=xs[:], in_=xt[mt].rearrange("a b c -> b a c"))
                for ms in range(MS):
                    pss = []
                    for ns in range(NS):
                        ps_t = pp.tile([128, NSW], f32, tag="ps",
                                       name=f"ps_{mt}_{ms}_{ns}")
                        pss.append(ps_t)
                    for kt in range(KT):
                        lhs = xs[:, kt, ms * 128:(ms + 1) * 128]
                        for ns in range(NS):
                            nc.tensor.matmul(pss[ns][:], lhs,
                                             wq[:, kt, ns * NSW:(ns + 1) * NSW],
                                             start=(kt == 0), stop=(kt == KT - 1))
                    ot = otp.tile([128, N], f32)
                    for ns in range(NS):
                        nc.vector.tensor_scalar_mul(ot[:, ns * NSW:(ns + 1) * NSW],
                                                    pss[ns][:], so128[:])
                    r0 = mt * MSLAB + ms * 128
                    nc.sync.dma_start(out=out[r0:r0 + 128, :], in_=ot[:])

    nc.compile()
    return nc


def prep_inputs(x, weight, scale, cores=CORES):
    """Host marshaling: returns per-core input maps."""
    x = np.asarray(x, dtype=np.float32)
    weight = np.asarray(weight, dtype=np.float32)
    scale = np.asarray(scale, dtype=np.float32)
    M, K, N = M_TOK, DIN, N_SHARD
    KT = K // 128
    MT = M // MSLAB

    xf = x.reshape(M, K)
    # [mt, kt, k, m] with value x[mt*MSLAB+m, kt*128+k], bf16
    xtile = np.ascontiguousarray(
        xf.reshape(MT, MSLAB, KT, 128).transpose(0, 2, 3, 1)
    ).astype(ml_dtypes.bfloat16)
    # [c, kt, k, n] with value weight[c*N+n, kt*128+k], f32
    wtile = np.ascontiguousarray(
        weight.reshape(cores, N, KT, 128).transpose(0, 2, 3, 1)
    )
    return [{"xt": xtile, "wt": wtile[c], "scale": scale} for c in range(cores)]


_NC_CACHE = {}


def kernel(x, weight, scale):
    import os
    from concourse.bass_utils import run_bass_kernel_spmd

    if os.environ.get("KERNEL_LDW_OPT", "") == "1":
        _enable_ldw_opt()
    if "nc" not in _NC_CACHE:
        _NC_CACHE["nc"] = build_nc()
    nc = _NC_CACHE["nc"]

    in_maps = prep_inputs(x, weight, scale)

    trace = os.environ.get("KERNEL_TRACE", "") == "1"
    kw = {}
    if trace:
        kw = dict(trace=True, trace_cores=[0])
    res = run_bass_kernel_spmd(nc, in_maps, core_ids=list(range(CORES)), **kw)
    _NC_CACHE["last_result"] = res

    outs = [res.results[c]["out"] for c in range(CORES)]
    full = np.concatenate(outs, axis=1).reshape(B, S, DOUT)
    return full


# revision 6
# speedup vs baseline: 1.5524x; 1.5524x over previous
"""BitLinear (ternary-quantized linear) kernel for 8 Trainium2 NeuronCores.

Reference computation:
    w_mean = mean(|W|)                       (global scalar over the full W)
    W_q    = clip(round(W / (w_mean+1e-5)), -1, 1)   in {-1, 0, 1}
    out    = x @ (W_q * w_mean * scale[0]).T

Sharding (column parallel): W is split along out_features across the 8 cores
(2048 rows each); x is replicated; each core produces out[:, shard] and the
host concatenates shards.

Device-side work per core:
  Phase A: partial sum of |W_shard| (DVE reduce) -> cross-partition reduce on
           GpSimd -> AllReduce(add) over the 8 cores -> w_mean, threshold
           h = 0.5*(w_mean+1e-5), out-scale so = 0.5*w_mean*scale[0]
           (the extra 0.5 dequantizes the {-2,0,2} ternary encoding below).
  Phase B: quantize W -> ternary in {-2, 0, +2} bf16, resident in SBUF
           K-major [128k, KT, N].  The work is spread over three engines so
           it keeps pace with the second W DMA pass:
             even chunks: ACT sign(w-h), ACT sign(w+h), DVE add  -> {-2,0,2}
             odd  chunks: GpSimd (w>=h)*2, GpSimd (w<=-h)*2, DVE sub
  Phase C: tiled matmul: stationary = x^T tiles (bf16), moving = W_q strips,
           fp32 PSUM accumulation over K, eviction scales by so on DVE.
           Matmuls of the first two PSUM groups overlap phase B.

Host marshaling: x is cast to bf16 and laid out K-major ([mt, kt, k, m]);
W is laid out K-major f32 per shard ([kt, k, n]). All FLOPs of the reference
computation (mean, quantize, matmul, scaling) happen on device.
"""

import numpy as np
import ml_dtypes

CORES = 8
B, S, DIN, DOUT = 4, 2048, 4096, 16384

_LDW_PATCHED = False


def _enable_ldw_opt():
    """Flip walrus --enable-ldw-opt to true: dedupes back-to-back LDWEIGHTS
    for matmuls that reuse the same stationary operand."""
    global _LDW_PATCHED
    if _LDW_PATCHED:
        return
    import concourse.bass_utils as bu
    orig = bu.run_command

    def patched(argv, **kw):
        argv = ["--enable-ldw-opt=true" if a == "--enable-ldw-opt=false" else a
                for a in argv]
        return orig(argv, **kw)

    bu.run_command = patched
    _LDW_PATCHED = True


M_TOK = B * S              # 8192 tokens
N_SHARD = DOUT // CORES    # 2048 out-features per core
MSLAB = 256                # tokens per x slab (2 stationary subtiles of 128)


def build_nc(cores=CORES, M=M_TOK, K=DIN, N=N_SHARD, n_weight_total=DOUT * DIN):
    """Build the (uncompiled-IR) Bass module for one SPMD core program."""
    import concourse.bacc as bacc
    import concourse.tile as tile
    import concourse.mybir as mybir
    import concourse.bass_isa as bass_isa

    f32 = mybir.dt.float32
    bf16 = mybir.dt.bfloat16
    X = mybir.AxisListType.X
    ADD = mybir.AluOpType.add
    MULT = mybir.AluOpType.mult
    ISGE = mybir.AluOpType.is_ge
    ISLE = mybir.AluOpType.is_le
    SIGN = mybir.ActivationFunctionType.Sign

    KT = K // 128
    NSW = 512               # matmul moving-strip width (PSUM bank limit)
    NS = N // NSW
    MT = M // MSLAB
    MS = MSLAB // 128
    QCH = 512               # quantize column chunk
    NQ = N // QCH
    inv_nw = 1.0 / float(n_weight_total)

    nc = bacc.Bacc("TRN2", target_bir_lowering=False, debug=False,
                   num_devices=cores)
    xt = nc.dram_tensor("xt", [MT, KT, 128, MSLAB], bf16, kind="ExternalInput")
    wt = nc.dram_tensor("wt", [KT, 128, N], f32, kind="ExternalInput")
    sc = nc.dram_tensor("scale", [1], f32, kind="ExternalInput")
    out = nc.dram_tensor("out", [M, N], f32, kind="ExternalOutput")

    with tile.TileContext(nc) as tc:
        with tc.tile_pool(name="const", bufs=1) as const, \
             tc.tile_pool(name="wqp", bufs=1) as wqp, \
             tc.tile_pool(name="wload", bufs=3) as wload, \
             tc.tile_pool(name="qtmp", bufs=2) as qtmp, \
             tc.tile_pool(name="xload", bufs=2) as xload, \
             tc.tile_pool(name="otp", bufs=2) as otp, \
             tc.tile_pool(name="pp", bufs=8 * 512 // NSW, space="PSUM") as pp, \
             tc.tile_pool(name="dram", bufs=1, space="DRAM") as dram:

            # ---------- Phase A: global mean(|W|) ----------
            partials = const.tile([128, KT], f32)
            for kt in range(KT):
                wa = wload.tile([128, N], f32, tag="w")
                eng = nc.sync if kt % 2 == 0 else nc.scalar
                eng.dma_start(out=wa[:], in_=wt[kt])
                nc.vector.tensor_reduce(partials[:, kt:kt + 1], wa[:], axis=X,
                                        op=ADD, apply_absolute_value=True)
            p1 = const.tile([128, 1], f32)
            nc.vector.tensor_reduce(p1[:], partials[:], axis=X, op=ADD)
            # cross-partition reduce on GpSimd: avoids an fp32 PE matmul,
            # which would disable fast-weight-load for the bf16 matmuls.
            tot_all = const.tile([128, 1], f32)
            nc.gpsimd.partition_all_reduce(tot_all[:], p1[:], 128,
                                           bass_isa.ReduceOp.add)
            tot_sb = tot_all[0:1, 0:1]

            cc_in = dram.tile([1, 1], f32)
            cc_space = "Shared" if cores > 4 else "Local"
            cc_out = dram.tile([1, 1], f32, addr_space=cc_space)
            nc.sync.dma_start(out=cc_in[:], in_=tot_sb[:])
            nc.gpsimd.collective_compute(
                "AllReduce", ADD,
                replica_groups=[list(range(cores))],
                ins=[cc_in.opt()], outs=[cc_out.opt()],
            )
            gsum = const.tile([1, 1], f32)
            nc.sync.dma_start(out=gsum[:], in_=cc_out[:])

            mean1 = const.tile([1, 1], f32)
            nc.scalar.mul(mean1[:], gsum[:], inv_nw)
            # h = 0.5 * (mean + 1e-5)
            me1 = const.tile([1, 1], f32)
            nc.vector.tensor_scalar(me1[:], mean1[:], 1e-5, None, ADD)
            h1 = const.tile([1, 1], f32)
            nc.vector.tensor_scalar(h1[:], me1[:], 0.5, None, MULT)
            sc_sb = const.tile([1, 1], f32)
            nc.sync.dma_start(out=sc_sb[:], in_=sc[:].rearrange("(a b) -> a b", b=1))
            # so = 0.5 * mean * scale  (the 0.5 dequantizes the {-2,0,2} code)
            mh1 = const.tile([1, 1], f32)
            nc.vector.tensor_scalar(mh1[:], mean1[:], 0.5, None, MULT)
            so1 = const.tile([1, 1], f32)
            nc.vector.tensor_mul(so1[:], mh1[:], sc_sb[:])
            h128 = const.tile([128, 1], f32)
            nc.gpsimd.partition_broadcast(h128[:], h1[:])
            h128n = const.tile([128, 1], f32)
            nc.vector.tensor_scalar(h128n[:], h128[:], -1.0, None, MULT)
            so128 = const.tile([128, 1], f32)
            nc.gpsimd.partition_broadcast(so128[:], so1[:])

            # ---------- Phase B: quantize W -> ternary {-2,0,2} bf16 ----------
            # Spread across ACT + GpSimd (compares) and DVE (combine) so the
            # per-tile cost stays below the 2.9us DMA pace and matmuls of the
            # open PSUM groups can interleave.
            wq = wqp.tile([128, KT, N], bf16)
            for kt in range(KT):
                wb = wload.tile([128, N], f32, tag="w")
                eng = nc.sync if kt % 2 == 0 else nc.gpsimd
                eng.dma_start(out=wb[:], in_=wt[kt])
                for c in range(NQ):
                    sl = slice(c * QCH, (c + 1) * QCH)
                    # 9 of every 16 chunks on ACT (sign pair), 7 on DVE
                    # (compare pair): balances ~1.4us/chunk ACT vs ~1.4us
                    # DVE-led + 0.4us DVE combine for ACT-led chunks.
                    act_led = ((kt * NQ + c) * 9) % 16 < 9
                    if act_led:
                        s1 = qtmp.tile([128, QCH], bf16, tag="qa")
                        nc.scalar.activation(s1[:], wb[:, sl], SIGN, bias=h128n[:])
                        s2 = qtmp.tile([128, QCH], bf16, tag="qb")
                        nc.scalar.activation(s2[:], wb[:, sl], SIGN, bias=h128[:])
                        nc.vector.tensor_add(wq[:, kt, sl], s1[:], s2[:])
                    else:
                        g1 = qtmp.tile([128, QCH], bf16, tag="qa")
                        nc.vector.tensor_scalar(g1[:], wb[:, sl], h128[:], 2.0,
                                                ISGE, MULT)
                        g2 = qtmp.tile([128, QCH], bf16, tag="qb")
                        nc.vector.tensor_scalar(g2[:], wb[:, sl], h128n[:], 2.0,
                                                ISLE, MULT)
                        nc.vector.tensor_sub(wq[:, kt, sl], g1[:], g2[:])

            # ---------- Phase C: out = x @ W_q^T * so ----------
            for mt in range(MT):
                xs = xload.tile([128, KT, MSLAB], bf16)
                nc.sync.dma_start(out=xs[:], in_=xt[mt].rearrange("a b c -> b a c"))
                for ms in range(MS):
                    pss = []
                    for ns in range(NS):
                        ps_t = pp.tile([128, NSW], f32, tag="ps",
                                       name=f"ps_{mt}_{ms}_{ns}")
                        pss.append(ps_t)
                    for kt in range(KT):
                        lhs = xs[:, kt, ms * 128:(ms + 1) * 128]
                        for ns in range(NS):
                            nc.tensor.matmul(pss[ns][:], lhs,
                                             wq[:, kt, ns * NSW:(ns + 1) * NSW],
                                             start=(kt == 0), stop=(kt == KT - 1))
                    ot = otp.tile([128, N], f32)
                    for ns in range(NS):
                        nc.vector.tensor_scalar_mul(ot[:, ns * NSW:(ns + 1) * NSW],
                                                    pss[ns][:], so128[:])
                    r0 = mt * MSLAB + ms * 128
                    nc.sync.dma_start(out=out[r0:r0 + 128, :], in_=ot[:])

    nc.compile()
    return nc


def prep_inputs(x, weight, scale, cores=CORES):
    """Host marshaling: returns per-core input maps."""
    x = np.asarray(x, dtype=np.float32)
    weight = np.asarray(weight, dtype=np.float32)
    scale = np.asarray(scale, dtype=np.float32)
    M, K, N = M_TOK, DIN, N_SHARD
    KT = K // 128
    MT = M // MSLAB

    xf = x.reshape(M, K)
    # [mt, kt, k, m] with value x[mt*MSLAB+m, kt*128+k], bf16
    xtile = np.ascontiguousarray(
        xf.reshape(MT, MSLAB, KT, 128).transpose(0, 2, 3, 1)
    ).astype(ml_dtypes.bfloat16)
    # [c, kt, k, n] with value weight[c*N+n, kt*128+k], f32
    wtile = np.ascontiguousarray(
        weight.reshape(cores, N, KT, 128).transpose(0, 2, 3, 1)
    )
    return [{"xt": xtile, "wt": wtile[c], "scale": scale} for c in range(cores)]


_NC_CACHE = {}


def kernel(x, weight, scale):
    import os
    from concourse.bass_utils import run_bass_kernel_spmd

    if os.environ.get("KERNEL_LDW_OPT", "") == "1":
        _enable_ldw_opt()
    if "nc" not in _NC_CACHE:
        _NC_CACHE["nc"] = build_nc()
    nc = _NC_CACHE["nc"]

    in_maps = prep_inputs(x, weight, scale)

    trace = os.environ.get("KERNEL_TRACE", "") == "1"
    kw = {}
    if trace:
        kw = dict(trace=True, trace_cores=[0])
    res = run_bass_kernel_spmd(nc, in_maps, core_ids=list(range(CORES)), **kw)
    _NC_CACHE["last_result"] = res

    outs = [res.results[c]["out"] for c in range(CORES)]
    full = np.concatenate(outs, axis=1).reshape(B, S, DOUT)
    return full
